# revision 14
# baseline (speedup 1.0000x reference)
"""CandidateFinder kernel for Trainium2 (8 NeuronCores, SPMD).

Problem: for each query i (per batch), find keys j where
  lsh_match(i,j) = any of 4 LSH hash buckets agree, AND
  trie_match(i,j) = all 12 sign bits of (batch -1) features agree.
Output [B, Sq, 64] int32: if count<=64, ascending candidate indices
right-aligned with -1 padding; if count>64, ascending top-64 by dot-sim.

Device strategy (v3): everything is ONE fp8 DoubleRow matmul per tile.
  - The pair predicate is linearized with NON-NEGATIVE encodings:
    compact one-hot LSH buckets (T occupied across the data, T <= 40)
    give lshdot in [0,4]; the 12 trie sign bits become 6 bit-PAIRS, each
    one-hot over 4 patterns (24 rows), agreements a in [0,6]. The score
      s = lshdot + 4*a   lies in [0, 28] and   match <=> s >= 25
    (a=6 and lshdot>=1), all in exact integer arithmetic.
  - THREE scores are packed per PSUM f32 via disjoint contraction blocks
    scaled 2^10 / 2^5 / 1 (the max fp8*fp8 exact power-of-2 product is
    128*128=2^14, bounding the field stack):  v = sum_f 2^(5f) s_f < 2^15.
    Blocks = 3 consecutive keys of one batch; fp8 DoubleRow contracts all
    3*(T+24) <= 192 rows in one pass at 1 column/cycle, so each PE cycle
    evaluates 3*128 pair predicates.
  - Per core: 512 queries x 2 batches = 8 stationary tiles of 128; keys
    padded to 4104 = 3*1368 pair-triple columns. PSUM f32 -> int16 SBUF
    (exact, v < 2^15; split across ACT/DVE) -> 2.7 MB/core DMA out. Host
    decodes 5-bit fields -> match bits -> candidate indices; count>64
    rows (never in practice) fall back to an exact host top-k.
Measured: see test.py (v1 dense-mask baseline was ~54 us).
"""

import numpy as np
from ml_dtypes import float8_e4m3

import concourse.bacc as bacc
import concourse.tile as tile
from concourse import mybir
from concourse.bass_utils import run_bass_kernel_spmd

B, S, D = 2, 4096, 12
H, BUCKETS, BW = 4, 32, 4.0
KMAX = 64
NCORES = 8
QPC = S // NCORES          # 512 query indices per core
NQT = QPC // 128           # 4 query tiles per core (x2 batches stationary)
SPAD = 4104                # keys padded to a multiple of 3
NKT = SPAD // 3            # 1368 key-triple columns per batch
TMAX = 40                  # max total occupied (hash,bucket) dims
KH = 96                    # DoubleRow half-K (2*96 = 192 contraction rows)
NPAIR = 6                  # trie sign-bit pairs
THR = 25                   # match <=> field score >= THR

# field scale split (q_scale * k_scale = 2^(5*f); pair rows carry 4x)
Q_LSH = (32.0, 8.0, 1.0)
K_LSH = (32.0, 4.0, 1.0)
Q_PAIR = (64.0, 16.0, 2.0)
K_PAIR = (64.0, 8.0, 2.0)

TRACE = False              # set True (module flag) to capture an NTFF trace
LAST_RESULTS = None

_nc_cache = None


def _build():
    global _nc_cache
    if _nc_cache is not None:
        return _nc_cache
    nc = bacc.Bacc()
    f8 = mybir.dt.float8e4
    i16 = mybir.dt.int16
    f32 = mybir.dt.float32
    dr = mybir.MatmulPerfMode.DoubleRow

    # stationary queries [b, qt] packed in one tensor; moving keys per batch
    qe_d = nc.dram_tensor("qe", [KH, 2, 2 * NQT, 128], f8,
                          kind="ExternalInput")
    ge_d = nc.dram_tensor("ge", [2, KH, 2, NKT], f8, kind="ExternalInput")
    out_d = nc.dram_tensor("out", [2, NQT, 128, NKT], i16,
                           kind="ExternalOutput")

    CHUNKS = (512, 512, 344)
    OFFS = (0, 512, 1024)

    with tile.TileContext(nc) as tc:
        with (
            tc.tile_pool(name="enc", bufs=1) as pool_e,
            tc.tile_pool(name="msk", bufs=10) as pool_m,
            tc.tile_pool(name="ps", bufs=6, space="PSUM") as pool_p,
        ):
            # all loads up front on the SP HWDGE queue, in the order the
            # PE consumes them (weights, then key chunks batch-major)
            qe_t = pool_e.tile([KH, 2, 2 * NQT, 128], f8, tag="qe")
            nc.sync.dma_start(out=qe_t[:], in_=qe_d[:])
            ge_t = {}
            for c in range(3):
                gt_ = pool_e.tile([KH, 2, CHUNKS[c]], f8, tag=f"ge0{c}")
                nc.sync.dma_start(
                    out=gt_[:],
                    in_=ge_d[0][:, :, OFFS[c]:OFFS[c] + CHUNKS[c]])
                ge_t[0, c] = gt_[:]
            gt_ = pool_e.tile([KH, 2, NKT], f8, tag="ge1")
            nc.sync.dma_start(out=gt_[:], in_=ge_d[1][:])
            for c in range(3):
                ge_t[1, c] = gt_[:, :, OFFS[c]:OFFS[c] + CHUNKS[c]]

            # tile-outer order: each output tile completes after 3 chunks so
            # its out-DMA issues early; outs split over two DMA queues
            k = 0
            for b in range(2):
                for t in range(NQT):
                    mt = pool_m.tile([128, NKT], i16, tag="m",
                                     name=f"m_{b}_{t}")
                    for c in range(3):
                        ch, o = CHUNKS[c], OFFS[c]
                        ps = pool_p.tile([128, ch], f32)
                        nc.tensor.matmul(
                            ps[:],
                            lhsT=qe_t[:, :, b * NQT + t],
                            rhs=ge_t[b, c],
                            start=True, stop=True, perf_mode=dr)
                        half = mt[:, o:o + ch]
                        # split the convert across ACT (faster) and DVE
                        if k % 2 == 0:
                            nc.scalar.copy(out=half, in_=ps[:])
                        else:
                            nc.vector.tensor_scalar(
                                out=half, in0=ps[:], scalar1=0.0,
                                scalar2=None, op0=mybir.AluOpType.add)
                        k += 1
                    if t % 2 == 0:
                        nc.sync.dma_start(out=out_d[b, t], in_=mt[:])
                    else:
                        nc.gpsimd.dma_start(out=out_d[b, t], in_=mt[:])

    nc.compile()
    _nc_cache = nc
    return nc


def _hashes(x, proj):
    # mirror: floor((x @ lsh_proj) / BW).astype(int32) % BUCKETS
    d = x.astype(np.float32) @ proj.astype(np.float32)
    return np.floor(d / BW).astype(np.int32) % BUCKETS


def _prep(q, k, proj):
    """Compact one-hot LSH + sign-pair encodings (all non-negative)."""
    qh = _hashes(q, proj)                       # [B,S,4]
    kh = _hashes(k, proj)
    sq = np.where(q[-1] > 0, np.float32(1.0), np.float32(-1.0))   # [S,12]
    sk = np.where(k[-1] > 0, np.float32(1.0), np.float32(-1.0))

    # compact remap of occupied (hash, bucket) pairs
    bases = []
    T = 0
    for h in range(H):
        occ = np.unique(np.concatenate(
            [qh[:, :, h].ravel(), kh[:, :, h].ravel()]))
        remap = np.full(BUCKETS, -1, np.int64)
        remap[occ] = T + np.arange(len(occ))
        bases.append(remap)
        T += len(occ)
    assert T <= TMAX, f"too many occupied buckets: {T}"
    nrow = T + 2 * NPAIR * 2  # T lsh rows + 6 pairs x 4 patterns

    # unscaled encodings [nrow, B, S]: lsh one-hot rows, then pair one-hots
    enc_q = np.zeros((nrow, B, S), np.float32)
    enc_k = np.zeros((nrow, B, S), np.float32)
    ar = np.arange(S)
    for b in range(B):
        for h in range(H):
            enc_q[bases[h][qh[b, :, h]], b, ar] = 1.0
            enc_k[bases[h][kh[b, :, h]], b, ar] = 1.0
    # sign-bit pairs (from batch B-1, shared across both batches)
    pq = (sq > 0).astype(np.int64)              # [S, 12]
    pk = (sk > 0).astype(np.int64)
    for p in range(NPAIR):
        code_q = pq[:, 2 * p] * 2 + pq[:, 2 * p + 1]   # [S] in 0..3
        code_k = pk[:, 2 * p] * 2 + pk[:, 2 * p + 1]
        enc_q[T + 4 * p + code_q, :, ar] = 1.0
        enc_k[T + 4 * p + code_k, :, ar] = 1.0

    return qh, kh, sq, sk, enc_q, enc_k, T


def _device_inputs(enc_q, enc_k, T):
    nrow = enc_q.shape[0]
    qe = np.zeros((2 * KH, 2 * NQT, NCORES, 128), np.float32)
    ge = np.zeros((2, 2 * KH, NKT), np.float32)
    encq_r = enc_q.reshape(nrow, B, NCORES, NQT, 128)
    enck_pad = np.zeros((nrow, B, SPAD), np.float32)
    enck_pad[:, :, :S] = enc_k
    for f in range(3):
        # DoubleRow pre-adds products of rows (d, d+KH) in reduced
        # precision, so both rows of a pair must carry the SAME block
        # scale: block f owns rows [32f, 32f+32) in each half.
        idx = np.where(np.arange(nrow) < 32,
                       32 * f + np.arange(nrow),
                       2 * KH // 2 + 32 * f + np.arange(nrow) - 32)
        for b in range(2):
            for t in range(NQT):
                blk = encq_r[:, b, :, t] * Q_LSH[f]
                blk[T:] *= Q_PAIR[f] / Q_LSH[f]
                qe[idx, b * NQT + t] = blk
            kblk = enck_pad[:, b, f::3] * K_LSH[f]
            kblk[T:] *= K_PAIR[f] / K_LSH[f]
            ge[b, idx] = kblk
    # [2*KH, ...] -> [KH, 2, ...] with row d = i*KH + p
    qe = qe.reshape(2, KH, 2 * NQT, NCORES, 128).transpose(1, 0, 2, 3, 4)
    ge = ge.reshape(2, 2, KH, NKT).transpose(0, 2, 1, 3)
    return (np.ascontiguousarray(qe).astype(float8_e4m3),
            np.ascontiguousarray(ge).astype(float8_e4m3))


def _mask_row(b, i, qh, kh, sq, sk):
    lsh = (qh[b, i][None, :] == kh[b]).any(-1)                  # [S]
    trie = (sq[i][None, :] == sk).all(-1)                       # [S]
    return lsh & trie


def _topk_row(q, k, b, i, maskrow):
    sims = q[b, i].astype(np.float32) @ k[b].astype(np.float32).T
    vals = np.where(maskrow, sims, -np.inf)
    top = np.argsort(-vals, kind="stable")[:KMAX]               # jax top_k tiebreak
    return np.sort(top).astype(np.int32)


def _ensure_ntff_hook():
    """The container's antenv stub lacks axon_hooks; synthesize it from the
    boot module's ctypes NTFF helper so trace=True can capture HW timings."""
    import sys
    import types
    try:
        from antenv.axon_hooks import get_axon_ntff_profile_hook  # noqa: F401
        return
    except ImportError:
        pass
    from trn_agent_boot.trn_boot import _ntff_profile_via_ctypes
    hook = _ntff_profile_via_ctypes("/opt/axon/libaxon_pjrt.so")
    mod = types.ModuleType("antenv.axon_hooks")
    state = {"hook": hook}
    mod.get_axon_ntff_profile_hook = lambda: state["hook"]
    mod.set_axon_ntff_profile_hook = lambda h: state.update(hook=h)
    import antenv
    antenv.axon_hooks = mod
    sys.modules["antenv.axon_hooks"] = mod


def kernel(**inputs):
    global LAST_RESULTS
    q = np.asarray(inputs["query_features_up"], np.float32)
    k = np.asarray(inputs["key_features_up"], np.float32)
    proj = np.asarray(inputs["lsh_proj"], np.float32)

    qh, kh, sq, sk, enc_q, enc_k, T = _prep(q, k, proj)
    qe, ge = _device_inputs(enc_q, enc_k, T)

    nc = _build()
    in_maps = [{"qe": np.ascontiguousarray(qe[:, :, :, c]), "ge": ge}
               for c in range(NCORES)]
    if TRACE:
        _ensure_ntff_hook()
    res = run_bass_kernel_spmd(
        nc, in_maps, core_ids=list(range(NCORES)), trace=TRACE
    )
    LAST_RESULTS = res

    # decode packed scores -> bool match grid [B, Sq, Sk]
    match = np.empty((B, S, S), np.bool_)
    for c in range(NCORES):
        v = res.results[c]["out"].astype(np.int32)   # [2, NQT, 128, NKT]
        v = v.reshape(2, QPC, NKT)                   # [b, query, triple]
        grid = np.empty((2, QPC, SPAD), np.bool_)
        for f in range(3):
            grid[:, :, f::3] = ((v >> (5 * (2 - f))) & 31) >= THR
        match[:, c * QPC:(c + 1) * QPC, :] = grid[:, :, :S]
        del v, grid

    cb, cq, ci = np.nonzero(match)
    rowid = cb.astype(np.int64) * S + cq
    counts = np.bincount(rowid, minlength=B * S)
    starts = np.concatenate(([0], np.cumsum(counts)))[:-1]
    ranks = np.arange(len(ci)) - starts[rowid]

    out = np.full((B * S, KMAX), -1, np.int32)
    cnt_row = counts[rowid]
    ok = cnt_row <= KMAX
    out[rowid[ok], (KMAX - cnt_row + ranks)[ok]] = ci[ok]

    # exact host fallback for count > KMAX rows (never happens in practice)
    for r in np.nonzero(counts > KMAX)[0]:
        b, i = divmod(int(r), S)
        mrow = _mask_row(b, i, qh, kh, sq, sk)
        out[r] = _topk_row(q, k, b, i, mrow)

    return out.reshape(B, S, KMAX)


# revision 17
# speedup vs baseline: 1.3178x; 1.3178x over previous
"""CandidateFinder kernel for Trainium2 (8 NeuronCores, SPMD).

Problem: for each query i (per batch), find keys j where
  lsh_match(i,j) = any of 4 LSH hash buckets agree, AND
  trie_match(i,j) = all 12 sign bits of (batch -1) features agree.
Output [B, Sq, 64] int32: if count<=64, ascending candidate indices
right-aligned with -1 padding; if count>64, ascending top-64 by dot-sim.

Device strategy (v3): everything is ONE fp8 DoubleRow matmul per tile.
  - The pair predicate is linearized with NON-NEGATIVE encodings:
    compact one-hot LSH buckets (T occupied across the data, T <= 40)
    give lshdot in [0,4]; the 12 trie sign bits become 6 bit-PAIRS, each
    one-hot over 4 patterns (24 rows), agreements a in [0,6]. The score
      s = lshdot + 4*a   lies in [0, 28] and   match <=> s >= 25
    (a=6 and lshdot>=1), all in exact integer arithmetic.
  - THREE scores are packed per PSUM f32 via disjoint contraction blocks
    scaled 2^10 / 2^5 / 1 (the max fp8*fp8 exact power-of-2 product is
    128*128=2^14, bounding the field stack):  v = sum_f 2^(5f) s_f < 2^15.
    Blocks = 3 consecutive keys of one batch; fp8 DoubleRow contracts all
    3*(T+24) <= 192 rows in one pass at 1 column/cycle, so each PE cycle
    evaluates 3*128 pair predicates.
  - Per core: 512 queries x 2 batches = 8 stationary tiles of 128; keys
    padded to 4104 = 3*1368 pair-triple columns. PSUM f32 -> int16 SBUF
    (exact, v < 2^15; split across ACT/DVE) -> 2.7 MB/core DMA out. Host
    decodes 5-bit fields -> match bits -> candidate indices; count>64
    rows (never in practice) fall back to an exact host top-k.
Measured: see test.py (v1 dense-mask baseline was ~54 us).
"""

import numpy as np
from ml_dtypes import float8_e4m3

import concourse.bacc as bacc
import concourse.tile as tile
from concourse import mybir
from concourse.bass_utils import run_bass_kernel_spmd

B, S, D = 2, 4096, 12
H, BUCKETS, BW = 4, 32, 4.0
KMAX = 64
NCORES = 8
QPC = S // NCORES          # 512 query indices per core
NQT = QPC // 128           # 4 query tiles per core (x2 batches stationary)
SPAD = 4104                # keys padded to a multiple of 3
NKT = SPAD // 3            # 1368 key-triple columns per batch
TMAX = 40                  # max total occupied (hash,bucket) dims
KH = 96                    # DoubleRow half-K (2*96 = 192 contraction rows)
NPAIR = 6                  # trie sign-bit pairs
THR = 25                   # match <=> field score >= THR

# field scale split (q_scale * k_scale = 2^(5*f); pair rows carry 4x)
Q_LSH = (32.0, 8.0, 1.0)
K_LSH = (32.0, 4.0, 1.0)
Q_PAIR = (64.0, 16.0, 2.0)
K_PAIR = (64.0, 8.0, 2.0)

TRACE = False              # set True (module flag) to capture an NTFF trace
LAST_RESULTS = None

_nc_cache = None


def _build():
    global _nc_cache
    if _nc_cache is not None:
        return _nc_cache
    nc = bacc.Bacc()
    f8 = mybir.dt.float8e4
    i16 = mybir.dt.int16
    f32 = mybir.dt.float32
    dr = mybir.MatmulPerfMode.DoubleRow

    # stationary queries [b, qt] packed in one tensor; moving keys per batch
    qe_d = nc.dram_tensor("qe", [KH, 2, 2 * NQT, 128], f8,
                          kind="ExternalInput")
    ge_d = nc.dram_tensor("ge", [2, KH, 2, NKT], f8, kind="ExternalInput")
    out_d = nc.dram_tensor("out", [2, NQT // 2, 128, 2, NKT], i16,
                           kind="ExternalOutput")

    CHUNKS = (512, 512, 344)
    OFFS = (0, 512, 1024)

    with tile.TileContext(nc) as tc:
        with (
            tc.tile_pool(name="enc", bufs=1) as pool_e,
            tc.tile_pool(name="msk", bufs=10) as pool_m,
            tc.tile_pool(name="ps", bufs=6, space="PSUM") as pool_p,
        ):
            # all loads up front on the SP HWDGE queue, in the order the
            # PE consumes them (weights, then key chunks batch-major)
            qe_t = pool_e.tile([KH, 2, 2 * NQT, 128], f8, tag="qe")
            nc.sync.dma_start(out=qe_t[:], in_=qe_d[:])
            ge_t = {}
            for c in range(3):
                gt_ = pool_e.tile([KH, 2, CHUNKS[c]], f8, tag=f"ge0{c}")
                nc.sync.dma_start(
                    out=gt_[:],
                    in_=ge_d[0][:, :, OFFS[c]:OFFS[c] + CHUNKS[c]])
                ge_t[0, c] = gt_[:]
            gt_ = pool_e.tile([KH, 2, NKT], f8, tag="ge1")
            nc.sync.dma_start(out=gt_[:], in_=ge_d[1][:])
            for c in range(3):
                ge_t[1, c] = gt_[:, :, OFFS[c]:OFFS[c] + CHUNKS[c]]

            # tile-outer order: each output tile completes after 3 chunks;
            # out-DMAs batch 2 tiles (longer descriptors amortize the
            # per-descriptor fetch overhead) and issue early on sync
            k = 0
            mt = None
            for b in range(2):
                for t in range(NQT):
                    if t % 2 == 0:
                        mt = pool_m.tile([128, 2, NKT], i16, tag="m",
                                         name=f"m_{b}_{t}")
                    for c in range(3):
                        ch, o = CHUNKS[c], OFFS[c]
                        ps = pool_p.tile([128, ch], f32)
                        nc.tensor.matmul(
                            ps[:],
                            lhsT=qe_t[:, :, b * NQT + t],
                            rhs=ge_t[b, c],
                            start=True, stop=True, perf_mode=dr)
                        half = mt[:, t % 2, o:o + ch]
                        # split the convert across ACT (faster) and DVE
                        if k % 2 == 0:
                            nc.scalar.copy(out=half, in_=ps[:])
                        else:
                            nc.vector.tensor_scalar(
                                out=half, in0=ps[:], scalar1=0.0,
                                scalar2=None, op0=mybir.AluOpType.add)
                        k += 1
                    if t % 2 == 1:
                        nc.sync.dma_start(out=out_d[b, t // 2], in_=mt[:])

    nc.compile()
    _nc_cache = nc
    return nc


def _hashes(x, proj):
    # mirror: floor((x @ lsh_proj) / BW).astype(int32) % BUCKETS
    d = x.astype(np.float32) @ proj.astype(np.float32)
    return np.floor(d / BW).astype(np.int32) % BUCKETS


def _prep(q, k, proj):
    """Compact one-hot LSH + sign-pair encodings (all non-negative)."""
    qh = _hashes(q, proj)                       # [B,S,4]
    kh = _hashes(k, proj)
    sq = np.where(q[-1] > 0, np.float32(1.0), np.float32(-1.0))   # [S,12]
    sk = np.where(k[-1] > 0, np.float32(1.0), np.float32(-1.0))

    # compact remap of occupied (hash, bucket) pairs
    bases = []
    T = 0
    for h in range(H):
        occ = np.unique(np.concatenate(
            [qh[:, :, h].ravel(), kh[:, :, h].ravel()]))
        remap = np.full(BUCKETS, -1, np.int64)
        remap[occ] = T + np.arange(len(occ))
        bases.append(remap)
        T += len(occ)
    assert T <= TMAX, f"too many occupied buckets: {T}"
    nrow = T + 2 * NPAIR * 2  # T lsh rows + 6 pairs x 4 patterns

    # unscaled encodings [nrow, B, S]: lsh one-hot rows, then pair one-hots
    enc_q = np.zeros((nrow, B, S), np.float32)
    enc_k = np.zeros((nrow, B, S), np.float32)
    ar = np.arange(S)
    for b in range(B):
        for h in range(H):
            enc_q[bases[h][qh[b, :, h]], b, ar] = 1.0
            enc_k[bases[h][kh[b, :, h]], b, ar] = 1.0
    # sign-bit pairs (from batch B-1, shared across both batches)
    pq = (sq > 0).astype(np.int64)              # [S, 12]
    pk = (sk > 0).astype(np.int64)
    for p in range(NPAIR):
        code_q = pq[:, 2 * p] * 2 + pq[:, 2 * p + 1]   # [S] in 0..3
        code_k = pk[:, 2 * p] * 2 + pk[:, 2 * p + 1]
        enc_q[T + 4 * p + code_q, :, ar] = 1.0
        enc_k[T + 4 * p + code_k, :, ar] = 1.0

    return qh, kh, sq, sk, enc_q, enc_k, T


def _device_inputs(enc_q, enc_k, T):
    nrow = enc_q.shape[0]
    qe = np.zeros((2 * KH, 2 * NQT, NCORES, 128), np.float32)
    ge = np.zeros((2, 2 * KH, NKT), np.float32)
    encq_r = enc_q.reshape(nrow, B, NCORES, NQT, 128)
    enck_pad = np.zeros((nrow, B, SPAD), np.float32)
    enck_pad[:, :, :S] = enc_k
    for f in range(3):
        # DoubleRow pre-adds products of rows (d, d+KH) in reduced
        # precision, so both rows of a pair must carry the SAME block
        # scale: block f owns rows [32f, 32f+32) in each half.
        idx = np.where(np.arange(nrow) < 32,
                       32 * f + np.arange(nrow),
                       2 * KH // 2 + 32 * f + np.arange(nrow) - 32)
        for b in range(2):
            for t in range(NQT):
                blk = encq_r[:, b, :, t] * Q_LSH[f]
                blk[T:] *= Q_PAIR[f] / Q_LSH[f]
                qe[idx, b * NQT + t] = blk
            kblk = enck_pad[:, b, f::3] * K_LSH[f]
            kblk[T:] *= K_PAIR[f] / K_LSH[f]
            ge[b, idx] = kblk
    # [2*KH, ...] -> [KH, 2, ...] with row d = i*KH + p
    qe = qe.reshape(2, KH, 2 * NQT, NCORES, 128).transpose(1, 0, 2, 3, 4)
    ge = ge.reshape(2, 2, KH, NKT).transpose(0, 2, 1, 3)
    return (np.ascontiguousarray(qe).astype(float8_e4m3),
            np.ascontiguousarray(ge).astype(float8_e4m3))


def _mask_row(b, i, qh, kh, sq, sk):
    lsh = (qh[b, i][None, :] == kh[b]).any(-1)                  # [S]
    trie = (sq[i][None, :] == sk).all(-1)                       # [S]
    return lsh & trie


def _topk_row(q, k, b, i, maskrow):
    sims = q[b, i].astype(np.float32) @ k[b].astype(np.float32).T
    vals = np.where(maskrow, sims, -np.inf)
    top = np.argsort(-vals, kind="stable")[:KMAX]               # jax top_k tiebreak
    return np.sort(top).astype(np.int32)


def _ensure_ntff_hook():
    """The container's antenv stub lacks axon_hooks; synthesize it from the
    boot module's ctypes NTFF helper so trace=True can capture HW timings."""
    import sys
    import types
    try:
        from antenv.axon_hooks import get_axon_ntff_profile_hook  # noqa: F401
        return
    except ImportError:
        pass
    from trn_agent_boot.trn_boot import _ntff_profile_via_ctypes
    hook = _ntff_profile_via_ctypes("/opt/axon/libaxon_pjrt.so")
    mod = types.ModuleType("antenv.axon_hooks")
    state = {"hook": hook}
    mod.get_axon_ntff_profile_hook = lambda: state["hook"]
    mod.set_axon_ntff_profile_hook = lambda h: state.update(hook=h)
    import antenv
    antenv.axon_hooks = mod
    sys.modules["antenv.axon_hooks"] = mod


def kernel(**inputs):
    global LAST_RESULTS
    q = np.asarray(inputs["query_features_up"], np.float32)
    k = np.asarray(inputs["key_features_up"], np.float32)
    proj = np.asarray(inputs["lsh_proj"], np.float32)

    qh, kh, sq, sk, enc_q, enc_k, T = _prep(q, k, proj)
    qe, ge = _device_inputs(enc_q, enc_k, T)

    nc = _build()
    in_maps = [{"qe": np.ascontiguousarray(qe[:, :, :, c]), "ge": ge}
               for c in range(NCORES)]
    if TRACE:
        _ensure_ntff_hook()
    res = run_bass_kernel_spmd(
        nc, in_maps, core_ids=list(range(NCORES)), trace=TRACE
    )
    LAST_RESULTS = res

    # decode packed scores -> bool match grid [B, Sq, Sk]
    match = np.empty((B, S, S), np.bool_)
    for c in range(NCORES):
        v = res.results[c]["out"].astype(np.int32)  # [2, NQT/2, 128, 2, NKT]
        v = v.transpose(0, 1, 3, 2, 4).reshape(2, QPC, NKT)  # [b, q, triple]
        grid = np.empty((2, QPC, SPAD), np.bool_)
        for f in range(3):
            grid[:, :, f::3] = ((v >> (5 * (2 - f))) & 31) >= THR
        match[:, c * QPC:(c + 1) * QPC, :] = grid[:, :, :S]
        del v, grid

    cb, cq, ci = np.nonzero(match)
    rowid = cb.astype(np.int64) * S + cq
    counts = np.bincount(rowid, minlength=B * S)
    starts = np.concatenate(([0], np.cumsum(counts)))[:-1]
    ranks = np.arange(len(ci)) - starts[rowid]

    out = np.full((B * S, KMAX), -1, np.int32)
    cnt_row = counts[rowid]
    ok = cnt_row <= KMAX
    out[rowid[ok], (KMAX - cnt_row + ranks)[ok]] = ci[ok]

    # exact host fallback for count > KMAX rows (never happens in practice)
    for r in np.nonzero(counts > KMAX)[0]:
        b, i = divmod(int(r), S)
        mrow = _mask_row(b, i, qh, kh, sq, sk)
        out[r] = _topk_row(q, k, b, i, mrow)

    return out.reshape(B, S, KMAX)


# revision 19
# speedup vs baseline: 1.3260x; 1.0062x over previous
"""CandidateFinder kernel for Trainium2 (8 NeuronCores, SPMD).

Problem: for each query i (per batch), find keys j where
  lsh_match(i,j) = any of 4 LSH hash buckets agree, AND
  trie_match(i,j) = all 12 sign bits of (batch -1) features agree.
Output [B, Sq, 64] int32: if count<=64, ascending candidate indices
right-aligned with -1 padding; if count>64, ascending top-64 by dot-sim.

Device strategy (v3): everything is ONE fp8 DoubleRow matmul per tile.
  - The pair predicate is linearized with NON-NEGATIVE encodings:
    compact one-hot LSH buckets (T occupied across the data, T <= 40)
    give lshdot in [0,4]; the 12 trie sign bits become 6 bit-PAIRS, each
    one-hot over 4 patterns (24 rows), agreements a in [0,6]. The score
      s = lshdot + 4*a   lies in [0, 28] and   match <=> s >= 25
    (a=6 and lshdot>=1), all in exact integer arithmetic.
  - THREE scores are packed per PSUM f32 via disjoint contraction blocks
    scaled 2^10 / 2^5 / 1 (the max fp8*fp8 exact power-of-2 product is
    128*128=2^14, bounding the field stack):  v = sum_f 2^(5f) s_f < 2^15.
    Blocks = 3 consecutive keys of one batch; fp8 DoubleRow contracts all
    3*(T+24) <= 192 rows in one pass at 1 column/cycle, so each PE cycle
    evaluates 3*128 pair predicates.
  - Per core: 512 queries x 2 batches = 8 stationary tiles of 128; keys
    padded to 4104 = 3*1368 pair-triple columns. PSUM f32 -> int16 SBUF
    (exact, v < 2^15; split across ACT/DVE) -> 2.7 MB/core DMA out. Host
    decodes 5-bit fields -> match bits -> candidate indices; count>64
    rows (never in practice) fall back to an exact host top-k.
Measured: see test.py (v1 dense-mask baseline was ~54 us).
"""

import numpy as np
from ml_dtypes import float8_e4m3

import concourse.bacc as bacc
import concourse.tile as tile
from concourse import mybir
from concourse.bass_utils import run_bass_kernel_spmd

B, S, D = 2, 4096, 12
H, BUCKETS, BW = 4, 32, 4.0
KMAX = 64
NCORES = 8
QPC = S // NCORES          # 512 query indices per core
NQT = QPC // 128           # 4 query tiles per core (x2 batches stationary)
SPAD = 4104                # keys padded to a multiple of 3
NKT = SPAD // 3            # 1368 key-triple columns per batch
TMAX = 60                  # max total occupied (hash,bucket) dims
HSMAX = 32                 # max rows per block-half (3 blocks * 2 halves)
NPAIR = 6                  # trie sign-bit pairs
THR = 25                   # match <=> field score >= THR

# field scale split (q_scale * k_scale = 2^(5*f); pair rows carry 4x)
Q_LSH = (32.0, 8.0, 1.0)
K_LSH = (32.0, 4.0, 1.0)
Q_PAIR = (64.0, 16.0, 2.0)
K_PAIR = (64.0, 8.0, 2.0)

TRACE = False              # set True (module flag) to capture an NTFF trace
LAST_RESULTS = None

_nc_cache = None


def _build(kh):
    global _nc_cache
    if _nc_cache is not None and _nc_cache[0] == kh:
        return _nc_cache[1]
    nc = bacc.Bacc()
    f8 = mybir.dt.float8e4
    i16 = mybir.dt.int16
    f32 = mybir.dt.float32
    dr = mybir.MatmulPerfMode.DoubleRow

    # stationary queries [b, qt] packed in one tensor; moving keys per batch
    qe_d = nc.dram_tensor("qe", [kh, 2, 2 * NQT, 128], f8,
                          kind="ExternalInput")
    ge_d = nc.dram_tensor("ge", [2, kh, 2, NKT], f8, kind="ExternalInput")
    out_d = nc.dram_tensor("out", [2, NQT // 2, 128, 2, NKT], i16,
                           kind="ExternalOutput")

    CHUNKS = (512, 512, 344)
    OFFS = (0, 512, 1024)

    with tile.TileContext(nc) as tc:
        with (
            tc.tile_pool(name="enc", bufs=1) as pool_e,
            tc.tile_pool(name="msk", bufs=10) as pool_m,
            tc.tile_pool(name="ps", bufs=6, space="PSUM") as pool_p,
        ):
            # all loads up front on the SP HWDGE queue, in the order the
            # PE consumes them (weights, then key chunks batch-major)
            qe_t = pool_e.tile([kh, 2, 2 * NQT, 128], f8, tag="qe")
            nc.sync.dma_start(out=qe_t[:, :, :NQT], in_=qe_d[:, :, :NQT])
            ge_t = {}
            ge_eng = (nc.sync, nc.scalar, nc.gpsimd)
            for c in range(3):
                gt_ = pool_e.tile([kh, 2, CHUNKS[c]], f8, tag=f"ge0{c}")
                ge_eng[c].dma_start(
                    out=gt_[:],
                    in_=ge_d[0][:, :, OFFS[c]:OFFS[c] + CHUNKS[c]])
                ge_t[0, c] = gt_[:]
            nc.sync.dma_start(out=qe_t[:, :, NQT:], in_=qe_d[:, :, NQT:])
            gt_ = pool_e.tile([kh, 2, NKT], f8, tag="ge1")
            nc.gpsimd.dma_start(out=gt_[:], in_=ge_d[1][:])
            for c in range(3):
                ge_t[1, c] = gt_[:, :, OFFS[c]:OFFS[c] + CHUNKS[c]]

            # tile-outer order: each output tile completes after 3 chunks;
            # out-DMAs batch 2 tiles (longer descriptors amortize the
            # per-descriptor fetch overhead) and issue early on sync
            k = 0
            mt = None
            for b in range(2):
                for t in range(NQT):
                    if t % 2 == 0:
                        mt = pool_m.tile([128, 2, NKT], i16, tag="m",
                                         name=f"m_{b}_{t}")
                    for c in range(3):
                        ch, o = CHUNKS[c], OFFS[c]
                        ps = pool_p.tile([128, ch], f32)
                        nc.tensor.matmul(
                            ps[:],
                            lhsT=qe_t[:, :, b * NQT + t],
                            rhs=ge_t[b, c],
                            start=True, stop=True, perf_mode=dr)
                        half = mt[:, t % 2, o:o + ch]
                        # split the convert across ACT (faster) and DVE
                        if k % 2 == 0:
                            nc.scalar.copy(out=half, in_=ps[:])
                        else:
                            nc.vector.tensor_scalar(
                                out=half, in0=ps[:], scalar1=0.0,
                                scalar2=None, op0=mybir.AluOpType.add)
                        k += 1
                    if t % 2 == 1:
                        nc.sync.dma_start(out=out_d[b, t // 2], in_=mt[:])

    nc.compile()
    _nc_cache = (kh, nc)
    return nc


def _hashes(x, proj):
    # mirror: floor((x @ lsh_proj) / BW).astype(int32) % BUCKETS
    d = x.astype(np.float32) @ proj.astype(np.float32)
    return np.floor(d / BW).astype(np.int32) % BUCKETS


def _prep(q, k, proj):
    """Compact one-hot LSH + sign-pair encodings (all non-negative)."""
    qh = _hashes(q, proj)                       # [B,S,4]
    kh = _hashes(k, proj)
    sq = np.where(q[-1] > 0, np.float32(1.0), np.float32(-1.0))   # [S,12]
    sk = np.where(k[-1] > 0, np.float32(1.0), np.float32(-1.0))

    # compact remap of occupied (hash, bucket) pairs
    bases = []
    T = 0
    for h in range(H):
        occ = np.unique(np.concatenate(
            [qh[:, :, h].ravel(), kh[:, :, h].ravel()]))
        remap = np.full(BUCKETS, -1, np.int64)
        remap[occ] = T + np.arange(len(occ))
        bases.append(remap)
        T += len(occ)
    assert T <= TMAX, f"too many occupied buckets: {T}"
    nrow = T + 2 * NPAIR * 2  # T lsh rows + 6 pairs x 4 patterns

    # unscaled encodings [nrow, B, S]: lsh one-hot rows, then pair one-hots
    enc_q = np.zeros((nrow, B, S), np.float32)
    enc_k = np.zeros((nrow, B, S), np.float32)
    ar = np.arange(S)
    for b in range(B):
        for h in range(H):
            enc_q[bases[h][qh[b, :, h]], b, ar] = 1.0
            enc_k[bases[h][kh[b, :, h]], b, ar] = 1.0
    # sign-bit pairs (from batch B-1, shared across both batches)
    pq = (sq > 0).astype(np.int64)              # [S, 12]
    pk = (sk > 0).astype(np.int64)
    for p in range(NPAIR):
        code_q = pq[:, 2 * p] * 2 + pq[:, 2 * p + 1]   # [S] in 0..3
        code_k = pk[:, 2 * p] * 2 + pk[:, 2 * p + 1]
        enc_q[T + 4 * p + code_q, :, ar] = 1.0
        enc_k[T + 4 * p + code_k, :, ar] = 1.0

    return qh, kh, sq, sk, enc_q, enc_k, T


def _device_inputs(enc_q, enc_k, T):
    nrow = enc_q.shape[0]
    hs = (nrow + 1) // 2       # rows per block-half
    kh = 3 * hs
    qe = np.zeros((2 * kh, 2 * NQT, NCORES, 128), np.float32)
    ge = np.zeros((2, 2 * kh, NKT), np.float32)
    encq_r = enc_q.reshape(nrow, B, NCORES, NQT, 128)
    enck_pad = np.zeros((nrow, B, SPAD), np.float32)
    enck_pad[:, :, :S] = enc_k
    for f in range(3):
        # DoubleRow pre-adds products of rows (d, d+kh) in reduced
        # precision, so both rows of a pair must carry the SAME block
        # scale: block f owns rows [hs*f, hs*(f+1)) in each half.
        idx = np.where(np.arange(nrow) < hs,
                       hs * f + np.arange(nrow),
                       kh + hs * f + np.arange(nrow) - hs)
        for b in range(2):
            for t in range(NQT):
                blk = encq_r[:, b, :, t] * Q_LSH[f]
                blk[T:] *= Q_PAIR[f] / Q_LSH[f]
                qe[idx, b * NQT + t] = blk
            kblk = enck_pad[:, b, f::3] * K_LSH[f]
            kblk[T:] *= K_PAIR[f] / K_LSH[f]
            ge[b, idx] = kblk
    # [2*kh, ...] -> [kh, 2, ...] with row d = i*kh + p
    qe = qe.reshape(2, kh, 2 * NQT, NCORES, 128).transpose(1, 0, 2, 3, 4)
    ge = ge.reshape(2, 2, kh, NKT).transpose(0, 2, 1, 3)
    return (np.ascontiguousarray(qe).astype(float8_e4m3),
            np.ascontiguousarray(ge).astype(float8_e4m3))


def _mask_row(b, i, qh, kh, sq, sk):
    lsh = (qh[b, i][None, :] == kh[b]).any(-1)                  # [S]
    trie = (sq[i][None, :] == sk).all(-1)                       # [S]
    return lsh & trie


def _topk_row(q, k, b, i, maskrow):
    sims = q[b, i].astype(np.float32) @ k[b].astype(np.float32).T
    vals = np.where(maskrow, sims, -np.inf)
    top = np.argsort(-vals, kind="stable")[:KMAX]               # jax top_k tiebreak
    return np.sort(top).astype(np.int32)


def _ensure_ntff_hook():
    """The container's antenv stub lacks axon_hooks; synthesize it from the
    boot module's ctypes NTFF helper so trace=True can capture HW timings."""
    import sys
    import types
    try:
        from antenv.axon_hooks import get_axon_ntff_profile_hook  # noqa: F401
        return
    except ImportError:
        pass
    from trn_agent_boot.trn_boot import _ntff_profile_via_ctypes
    hook = _ntff_profile_via_ctypes("/opt/axon/libaxon_pjrt.so")
    mod = types.ModuleType("antenv.axon_hooks")
    state = {"hook": hook}
    mod.get_axon_ntff_profile_hook = lambda: state["hook"]
    mod.set_axon_ntff_profile_hook = lambda h: state.update(hook=h)
    import antenv
    antenv.axon_hooks = mod
    sys.modules["antenv.axon_hooks"] = mod


def kernel(**inputs):
    global LAST_RESULTS
    q = np.asarray(inputs["query_features_up"], np.float32)
    k = np.asarray(inputs["key_features_up"], np.float32)
    proj = np.asarray(inputs["lsh_proj"], np.float32)

    qh, kh, sq, sk, enc_q, enc_k, T = _prep(q, k, proj)
    qe, ge = _device_inputs(enc_q, enc_k, T)

    nc = _build(qe.shape[0])
    in_maps = [{"qe": np.ascontiguousarray(qe[:, :, :, c]), "ge": ge}
               for c in range(NCORES)]
    if TRACE:
        _ensure_ntff_hook()
    res = run_bass_kernel_spmd(
        nc, in_maps, core_ids=list(range(NCORES)), trace=TRACE
    )
    LAST_RESULTS = res

    # decode packed scores -> bool match grid [B, Sq, Sk]
    match = np.empty((B, S, S), np.bool_)
    for c in range(NCORES):
        v = res.results[c]["out"].astype(np.int32)  # [2, NQT/2, 128, 2, NKT]
        v = v.transpose(0, 1, 3, 2, 4).reshape(2, QPC, NKT)  # [b, q, triple]
        grid = np.empty((2, QPC, SPAD), np.bool_)
        for f in range(3):
            grid[:, :, f::3] = ((v >> (5 * (2 - f))) & 31) >= THR
        match[:, c * QPC:(c + 1) * QPC, :] = grid[:, :, :S]
        del v, grid

    cb, cq, ci = np.nonzero(match)
    rowid = cb.astype(np.int64) * S + cq
    counts = np.bincount(rowid, minlength=B * S)
    starts = np.concatenate(([0], np.cumsum(counts)))[:-1]
    ranks = np.arange(len(ci)) - starts[rowid]

    out = np.full((B * S, KMAX), -1, np.int32)
    cnt_row = counts[rowid]
    ok = cnt_row <= KMAX
    out[rowid[ok], (KMAX - cnt_row + ranks)[ok]] = ci[ok]

    # exact host fallback for count > KMAX rows (never happens in practice)
    for r in np.nonzero(counts > KMAX)[0]:
        b, i = divmod(int(r), S)
        mrow = _mask_row(b, i, qh, kh, sq, sk)
        out[r] = _topk_row(q, k, b, i, mrow)

    return out.reshape(B, S, KMAX)


# revision 21
# speedup vs baseline: 1.3742x; 1.0364x over previous
"""CandidateFinder kernel for Trainium2 (8 NeuronCores, SPMD).

Problem: for each query i (per batch), find keys j where
  lsh_match(i,j) = any of 4 LSH hash buckets agree, AND
  trie_match(i,j) = all 12 sign bits of (batch -1) features agree.
Output [B, Sq, 64] int32: if count<=64, ascending candidate indices
right-aligned with -1 padding; if count>64, ascending top-64 by dot-sim.

Device strategy (v3): everything is ONE fp8 DoubleRow matmul per tile.
  - The pair predicate is linearized with NON-NEGATIVE encodings:
    compact one-hot LSH buckets (T occupied across the data, T <= 40)
    give lshdot in [0,4]; the 12 trie sign bits become 6 bit-PAIRS, each
    one-hot over 4 patterns (24 rows), agreements a in [0,6]. The score
      s = lshdot + 4*a   lies in [0, 28] and   match <=> s >= 25
    (a=6 and lshdot>=1), all in exact integer arithmetic.
  - THREE scores are packed per PSUM f32 via disjoint contraction blocks
    scaled 2^10 / 2^5 / 1 (the max fp8*fp8 exact power-of-2 product is
    128*128=2^14, bounding the field stack):  v = sum_f 2^(5f) s_f < 2^15.
    Blocks = 3 consecutive keys of one batch; fp8 DoubleRow contracts all
    3*(T+24) <= 192 rows in one pass at 1 column/cycle, so each PE cycle
    evaluates 3*128 pair predicates.
  - Per core: 512 queries x 2 batches = 8 stationary tiles of 128; keys
    padded to 4104 = 3*1368 pair-triple columns. PSUM f32 -> int16 SBUF
    (exact, v < 2^15; split across ACT/DVE) -> 2.7 MB/core DMA out. Host
    decodes 5-bit fields -> match bits -> candidate indices; count>64
    rows (never in practice) fall back to an exact host top-k.
Measured: see test.py (v1 dense-mask baseline was ~54 us).
"""

import numpy as np
from ml_dtypes import float8_e4m3

import concourse.bacc as bacc
import concourse.tile as tile
from concourse import mybir
from concourse.bass_utils import run_bass_kernel_spmd

B, S, D = 2, 4096, 12
H, BUCKETS, BW = 4, 32, 4.0
KMAX = 64
NCORES = 8
QPC = S // NCORES          # 512 query indices per core
NQT = QPC // 128           # 4 query tiles per core (x2 batches stationary)
SPAD = 4104                # keys padded to a multiple of 3
NKT = SPAD // 3            # 1368 key-triple columns per batch
TMAX = 60                  # max total occupied (hash,bucket) dims
HSMAX = 32                 # max rows per block-half (3 blocks * 2 halves)
NPAIR = 6                  # trie sign-bit pairs
THR = 25                   # match <=> field score >= THR

# field scale split (q_scale * k_scale = 2^(5*f); pair rows carry 4x)
Q_LSH = (32.0, 8.0, 1.0)
K_LSH = (32.0, 4.0, 1.0)
Q_PAIR = (64.0, 16.0, 2.0)
K_PAIR = (64.0, 8.0, 2.0)

TRACE = False              # set True (module flag) to capture an NTFF trace
LAST_RESULTS = None

_nc_cache = None


def _build(kh):
    global _nc_cache
    if _nc_cache is not None and _nc_cache[0] == kh:
        return _nc_cache[1]
    nc = bacc.Bacc()
    f8 = mybir.dt.float8e4
    i16 = mybir.dt.int16
    f32 = mybir.dt.float32
    dr = mybir.MatmulPerfMode.DoubleRow

    # stationary queries [b, qt] packed in one tensor; moving keys per batch
    qe_d = nc.dram_tensor("qe", [kh, 2, 2 * NQT, 128], f8,
                          kind="ExternalInput")
    # batch-0 key chunks are separate contiguous tensors (big descriptors)
    ge0_d = [nc.dram_tensor(f"ge0{c}", [kh, 2, 512 if c < 2 else 344], f8,
                            kind="ExternalInput") for c in range(3)]
    ge1_d = nc.dram_tensor("ge1", [kh, 2, NKT], f8, kind="ExternalInput")
    out_d = nc.dram_tensor("out", [2, NQT // 2, 128, 2, NKT], i16,
                           kind="ExternalOutput")

    CHUNKS = (512, 512, 344)
    OFFS = (0, 512, 1024)

    with tile.TileContext(nc) as tc:
        with (
            tc.tile_pool(name="enc", bufs=1) as pool_e,
            tc.tile_pool(name="msk", bufs=10) as pool_m,
            tc.tile_pool(name="ps", bufs=6, space="PSUM") as pool_p,
        ):
            # all loads up front on the SP HWDGE queue, in the order the
            # PE consumes them (weights, then key chunks batch-major)
            qe_t = pool_e.tile([kh, 2, 2 * NQT, 128], f8, tag="qe")
            nc.sync.dma_start(out=qe_t[:], in_=qe_d[:])
            ge_t = {}
            for c in range(3):
                gt_ = pool_e.tile([kh, 2, CHUNKS[c]], f8, tag=f"ge0{c}")
                nc.sync.dma_start(out=gt_[:], in_=ge0_d[c][:])
                ge_t[0, c] = gt_[:]
            gt_ = pool_e.tile([kh, 2, NKT], f8, tag="ge1")
            nc.gpsimd.dma_start(out=gt_[:], in_=ge1_d[:])
            for c in range(3):
                ge_t[1, c] = gt_[:, :, OFFS[c]:OFFS[c] + CHUNKS[c]]

            # tile-outer order: each output tile completes after 3 chunks;
            # out-DMAs batch 2 tiles (longer descriptors amortize the
            # per-descriptor fetch overhead) and issue early on sync
            k = 0
            mt = None
            for b in range(2):
                for t in range(NQT):
                    if t % 2 == 0:
                        mt = pool_m.tile([128, 2, NKT], i16, tag="m",
                                         name=f"m_{b}_{t}")
                    for c in range(3):
                        ch, o = CHUNKS[c], OFFS[c]
                        ps = pool_p.tile([128, ch], f32)
                        nc.tensor.matmul(
                            ps[:],
                            lhsT=qe_t[:, :, b * NQT + t],
                            rhs=ge_t[b, c],
                            start=True, stop=True, perf_mode=dr)
                        half = mt[:, t % 2, o:o + ch]
                        # split the convert across ACT (faster) and DVE
                        if k % 2 == 0:
                            nc.scalar.copy(out=half, in_=ps[:])
                        else:
                            nc.vector.tensor_scalar(
                                out=half, in0=ps[:], scalar1=0.0,
                                scalar2=None, op0=mybir.AluOpType.add)
                        k += 1
                    if t % 2 == 1:
                        nc.sync.dma_start(out=out_d[b, t // 2], in_=mt[:])

    nc.compile()
    _nc_cache = (kh, nc)
    return nc


def _hashes(x, proj):
    # mirror: floor((x @ lsh_proj) / BW).astype(int32) % BUCKETS
    d = x.astype(np.float32) @ proj.astype(np.float32)
    return np.floor(d / BW).astype(np.int32) % BUCKETS


def _prep(q, k, proj):
    """Compact one-hot LSH + sign-pair encodings (all non-negative)."""
    qh = _hashes(q, proj)                       # [B,S,4]
    kh = _hashes(k, proj)
    sq = np.where(q[-1] > 0, np.float32(1.0), np.float32(-1.0))   # [S,12]
    sk = np.where(k[-1] > 0, np.float32(1.0), np.float32(-1.0))

    # compact remap of occupied (hash, bucket) pairs
    bases = []
    T = 0
    for h in range(H):
        occ = np.unique(np.concatenate(
            [qh[:, :, h].ravel(), kh[:, :, h].ravel()]))
        remap = np.full(BUCKETS, -1, np.int64)
        remap[occ] = T + np.arange(len(occ))
        bases.append(remap)
        T += len(occ)
    assert T <= TMAX, f"too many occupied buckets: {T}"
    nrow = T + 2 * NPAIR * 2  # T lsh rows + 6 pairs x 4 patterns

    # unscaled encodings [nrow, B, S]: lsh one-hot rows, then pair one-hots
    enc_q = np.zeros((nrow, B, S), np.float32)
    enc_k = np.zeros((nrow, B, S), np.float32)
    ar = np.arange(S)
    for b in range(B):
        for h in range(H):
            enc_q[bases[h][qh[b, :, h]], b, ar] = 1.0
            enc_k[bases[h][kh[b, :, h]], b, ar] = 1.0
    # sign-bit pairs (from batch B-1, shared across both batches)
    pq = (sq > 0).astype(np.int64)              # [S, 12]
    pk = (sk > 0).astype(np.int64)
    for p in range(NPAIR):
        code_q = pq[:, 2 * p] * 2 + pq[:, 2 * p + 1]   # [S] in 0..3
        code_k = pk[:, 2 * p] * 2 + pk[:, 2 * p + 1]
        enc_q[T + 4 * p + code_q, :, ar] = 1.0
        enc_k[T + 4 * p + code_k, :, ar] = 1.0

    return qh, kh, sq, sk, enc_q, enc_k, T


def _device_inputs(enc_q, enc_k, T):
    nrow = enc_q.shape[0]
    hs = (nrow + 1) // 2       # rows per block-half
    kh = 3 * hs
    qe = np.zeros((2 * kh, 2 * NQT, NCORES, 128), np.float32)
    ge = np.zeros((2, 2 * kh, NKT), np.float32)
    encq_r = enc_q.reshape(nrow, B, NCORES, NQT, 128)
    enck_pad = np.zeros((nrow, B, SPAD), np.float32)
    enck_pad[:, :, :S] = enc_k
    for f in range(3):
        # DoubleRow pre-adds products of rows (d, d+kh) in reduced
        # precision, so both rows of a pair must carry the SAME block
        # scale: block f owns rows [hs*f, hs*(f+1)) in each half.
        idx = np.where(np.arange(nrow) < hs,
                       hs * f + np.arange(nrow),
                       kh + hs * f + np.arange(nrow) - hs)
        for b in range(2):
            for t in range(NQT):
                blk = encq_r[:, b, :, t] * Q_LSH[f]
                blk[T:] *= Q_PAIR[f] / Q_LSH[f]
                qe[idx, b * NQT + t] = blk
            kblk = enck_pad[:, b, f::3] * K_LSH[f]
            kblk[T:] *= K_PAIR[f] / K_LSH[f]
            ge[b, idx] = kblk
    # [2*kh, ...] -> [kh, 2, ...] with row d = i*kh + p
    qe = qe.reshape(2, kh, 2 * NQT, NCORES, 128).transpose(1, 0, 2, 3, 4)
    ge = ge.reshape(2, 2, kh, NKT).transpose(0, 2, 1, 3)
    return (np.ascontiguousarray(qe).astype(float8_e4m3),
            np.ascontiguousarray(ge).astype(float8_e4m3))


def _mask_row(b, i, qh, kh, sq, sk):
    lsh = (qh[b, i][None, :] == kh[b]).any(-1)                  # [S]
    trie = (sq[i][None, :] == sk).all(-1)                       # [S]
    return lsh & trie


def _topk_row(q, k, b, i, maskrow):
    sims = q[b, i].astype(np.float32) @ k[b].astype(np.float32).T
    vals = np.where(maskrow, sims, -np.inf)
    top = np.argsort(-vals, kind="stable")[:KMAX]               # jax top_k tiebreak
    return np.sort(top).astype(np.int32)


def _ensure_ntff_hook():
    """The container's antenv stub lacks axon_hooks; synthesize it from the
    boot module's ctypes NTFF helper so trace=True can capture HW timings."""
    import sys
    import types
    try:
        from antenv.axon_hooks import get_axon_ntff_profile_hook  # noqa: F401
        return
    except ImportError:
        pass
    from trn_agent_boot.trn_boot import _ntff_profile_via_ctypes
    hook = _ntff_profile_via_ctypes("/opt/axon/libaxon_pjrt.so")
    mod = types.ModuleType("antenv.axon_hooks")
    state = {"hook": hook}
    mod.get_axon_ntff_profile_hook = lambda: state["hook"]
    mod.set_axon_ntff_profile_hook = lambda h: state.update(hook=h)
    import antenv
    antenv.axon_hooks = mod
    sys.modules["antenv.axon_hooks"] = mod


def kernel(**inputs):
    global LAST_RESULTS
    q = np.asarray(inputs["query_features_up"], np.float32)
    k = np.asarray(inputs["key_features_up"], np.float32)
    proj = np.asarray(inputs["lsh_proj"], np.float32)

    qh, kh, sq, sk, enc_q, enc_k, T = _prep(q, k, proj)
    qe, ge = _device_inputs(enc_q, enc_k, T)

    nc = _build(qe.shape[0])
    ge_chunks = {
        "ge00": np.ascontiguousarray(ge[0][:, :, :512]),
        "ge01": np.ascontiguousarray(ge[0][:, :, 512:1024]),
        "ge02": np.ascontiguousarray(ge[0][:, :, 1024:]),
        "ge1": np.ascontiguousarray(ge[1]),
    }
    in_maps = [{"qe": np.ascontiguousarray(qe[:, :, :, c]), **ge_chunks}
               for c in range(NCORES)]
    if TRACE:
        _ensure_ntff_hook()
    res = run_bass_kernel_spmd(
        nc, in_maps, core_ids=list(range(NCORES)), trace=TRACE
    )
    LAST_RESULTS = res

    # decode packed scores -> bool match grid [B, Sq, Sk]
    match = np.empty((B, S, S), np.bool_)
    for c in range(NCORES):
        v = res.results[c]["out"].astype(np.int32)  # [2, NQT/2, 128, 2, NKT]
        v = v.transpose(0, 1, 3, 2, 4).reshape(2, QPC, NKT)  # [b, q, triple]
        grid = np.empty((2, QPC, SPAD), np.bool_)
        for f in range(3):
            grid[:, :, f::3] = ((v >> (5 * (2 - f))) & 31) >= THR
        match[:, c * QPC:(c + 1) * QPC, :] = grid[:, :, :S]
        del v, grid

    cb, cq, ci = np.nonzero(match)
    rowid = cb.astype(np.int64) * S + cq
    counts = np.bincount(rowid, minlength=B * S)
    starts = np.concatenate(([0], np.cumsum(counts)))[:-1]
    ranks = np.arange(len(ci)) - starts[rowid]

    out = np.full((B * S, KMAX), -1, np.int32)
    cnt_row = counts[rowid]
    ok = cnt_row <= KMAX
    out[rowid[ok], (KMAX - cnt_row + ranks)[ok]] = ci[ok]

    # exact host fallback for count > KMAX rows (never happens in practice)
    for r in np.nonzero(counts > KMAX)[0]:
        b, i = divmod(int(r), S)
        mrow = _mask_row(b, i, qh, kh, sq, sk)
        out[r] = _topk_row(q, k, b, i, mrow)

    return out.reshape(B, S, KMAX)
